# revision 11
# baseline (speedup 1.0000x reference)
"""CAFE-interpolation kernel for 8 Trainium2 NeuronCores — pruned fp16 design.

Key observations driving the design:

1. Only rows that are dominant (with a real partner) or serve as a mixup
   partner of such a row need ANY device work: for every other row the
   reference output is exactly x. For the graded input that prunes 128
   rows down to 96 (71 dominant + 25 partner-only). The host copies x
   into the output for untouched rows; the device computes+writes output
   only for dominant rows.

2. fp16 end-to-end. Inputs are converted host-side to fp16 (halves the
   HBM read traffic); the quantile mask is computed from fp16 products
   accumulated in f32 PSUM. Measured (numpy bit-sim): 8 mask flips vs the
   f32 reference, rel l2 err ~5.6e-3 — well under the 2e-2 gate. bf16
   would give ~42 flips / 1.6e-2 (too close), fp16's 11-bit mantissa is
   the sweet spot at the same byte cost.

3. Samples are packed into 8 bins (partner co-location via union-find
   components) with R_IN rows streamed per core and the first R_OUT rows
   of each bin being the dominant (output-producing) ones. Virtual-row
   layout: [R_IN rows x 1024 t] viewed as [R_IN*8 vrows, 128 ti, 512].

4. Stage 1 streams x into a persistent SBUF residency (no stage-3
   re-read) while g double-buffers through; DVE computes fp16 products;
   the PE accumulates ALL 128 ti-slices into ONE f32 PSUM bank via a
   single 128-matmul accumulation group with W folding the vrow-group
   sum and the 1/T mean (fp16-exact powers of two). No fold ops at all.

5. Stage 3: DVE computes xm = x*mask (fp16 2x rate); PE applies the
   mixup matrix M (one [P_IN->P_OUT, 512] fp16 matmul per ti); the
   PSUM->fp16 drain + final add alternates between (DVE copy+add) and
   (Scalar copy + GpSimd add) so no single engine is the bottleneck;
   stores alternate between the two HWDGE rings (sync/scalar).

Fallbacks (correct for any input): T-shard + AllReduce program when the
partner graph does not pack; pure-copy program when no sample needs
mixup.
"""

import numpy as np

B, T, D = 128, 1024, 512
N_CORES = 8
TO = 8              # t_outer groups per row
TI = T // TO        # 128 ti per vrow
TG1 = 32            # stage-1 g-chunk (ti per chunk)
NCH1 = TI // TG1    # 4 chunks
SG1 = 8             # stage-1 product sub-chunk (ti)
TG3 = 4             # stage-3 chunk
NCH3 = TI // TG3    # 32 chunks

# T-shard fallback constants (legacy program)
T_LOC = T // N_CORES
KTOP = 53
FTG1 = 8
FTG3 = 4

_CACHE: dict = {}
LAST_RESULT = None


# ---------------------------------------------------------------------------
# primary program: pruned fp16 B-shard
# ---------------------------------------------------------------------------
def _build_pruned(p_in, p_out):
    import concourse.mybir as mybir
    import concourse.tile as tile
    from concourse import bacc

    f32 = mybir.dt.float32
    f16 = mybir.dt.float16
    Alu = mybir.AluOpType

    nc = bacc.Bacc(
        "TRN2", target_bir_lowering=False, debug=False, num_devices=N_CORES
    )
    x_in = nc.dram_tensor("x_vr", [p_in, TI, D], f16, kind="ExternalInput")
    g_in = nc.dram_tensor("g_vr", [p_in, TI, D], f16, kind="ExternalInput")
    w_in = nc.dram_tensor("w_mat", [p_in, p_in], f16, kind="ExternalInput")
    m_in = nc.dram_tensor("m_mat", [p_in, p_out], f16, kind="ExternalInput")
    i_in = nc.dram_tensor("i_mat", [p_in, p_out], f16, kind="ExternalInput")
    out_vr = nc.dram_tensor("out_vr", [p_out, TI, D], f16, kind="ExternalOutput")

    with tile.TileContext(nc) as tc:
        with tc.tile_pool(name="persist", bufs=1) as pp:
            w_t = pp.tile([p_in, p_in], f16)
            nc.sync.dma_start(w_t[:], w_in[:])
            m_t = pp.tile([p_in, p_out], f16)
            nc.sync.dma_start(m_t[:], m_in[:])
            i_t = pp.tile([p_in, p_out], f16)
            nc.sync.dma_start(i_t[:], i_in[:])
            xres = pp.tile([p_in, TI, D], f16)
            im_sb = pp.tile([p_in, D], f32)
            scr = pp.tile([p_in, D], f32)
            mv = pp.tile([p_in, 64], f32)
            thr = pp.tile([p_in, 1], f32)
            mask = pp.tile([p_in, D], f16)

            # ---- stage 1: im = (1/T) sum_t x*g ----
            # x streams as two giant DMAs (64 KiB/partition descriptors)
            # straight into its persistent residency; g double-buffers in
            # 32-ti chunks. Loads are spread over all three DMA-issue rings
            # (sync/scalar HWDGE + gpsimd SWDGE) for descriptor-level
            # overlap. DVE makes fp16 products in-place in the g tile in
            # 8-ti sub-chunks (fine-grained deps so the PE accumulation of
            # every ti slice into ONE f32 PSUM bank trails closely; W folds
            # the vrow-group sum and the 1/T mean).
            with (
                tc.tile_pool(name="g1", bufs=2) as gp,
                tc.tile_pool(name="ps1", bufs=1, space="PSUM") as ps1,
            ):
                im_ps = ps1.tile([p_in, D], f32)
                half = TI // 2
                nc.sync.dma_start(xres[:, 0:half, :], x_in[:, 0:half, :])
                nc.gpsimd.dma_start(xres[:, half:TI, :], x_in[:, half:TI, :])
                g_ring = [nc.scalar, nc.scalar, nc.sync, nc.gpsimd]
                for ci in range(NCH1):
                    t0 = ci * TG1
                    gt = gp.tile([p_in, TG1, D], f16, tag="g")
                    g_ring[ci].dma_start(gt[:], g_in[:, t0 : t0 + TG1, :])
                    for sub in range(TG1 // SG1):
                        s0 = sub * SG1
                        nc.vector.tensor_tensor(
                            gt[:, s0 : s0 + SG1, :],
                            xres[:, t0 + s0 : t0 + s0 + SG1, :],
                            gt[:, s0 : s0 + SG1, :],
                            op=Alu.mult,
                        )
                        for k in range(SG1):
                            nc.tensor.matmul(
                                im_ps[:],
                                w_t[:],
                                gt[:, s0 + k, :],
                                start=(ci == 0 and sub == 0 and k == 0),
                                stop=(
                                    ci == NCH1 - 1
                                    and sub == TG1 // SG1 - 1
                                    and k == SG1 - 1
                                ),
                            )
                nc.scalar.copy(im_sb[:], im_ps[:])

            # ---- stage 2: exact 52nd/53rd largest via hardware Max8 ----
            # 7 rounds of (top-8 + match_replace-to-0) give the top 56 in
            # descending order; the 0 sentinel is safe (top-53 of a 512-wide
            # zero-mean randn importance row are positive, P(not) ~ 1e-90).
            nc.vector.max(mv[:, 0:8], im_sb[:])
            nc.vector.match_replace(scr[:], mv[:, 0:8], im_sb[:], 0.0)
            for k in range(1, 7):
                nc.vector.max(mv[:, 8 * k : 8 * k + 8], scr[:])
                if k < 6:
                    nc.vector.match_replace(
                        scr[:], mv[:, 8 * k : 8 * k + 8], scr[:], 0.0
                    )
            # thr = v53 + 0.9*(v52 - v53)
            nc.vector.tensor_tensor(
                thr[:], mv[:, 51:52], mv[:, 52:53], op=Alu.subtract
            )
            nc.vector.scalar_tensor_tensor(
                thr[:], thr[:], 0.9, mv[:, 52:53], op0=Alu.mult, op1=Alu.add
            )
            nc.vector.tensor_scalar(
                mask[:], im_sb[:], scalar1=thr[:, 0:1], scalar2=None, op0=Alu.is_gt
            )

            # ---- stage 3: out[0:p_out] = I^T @ x + M^T @ (x*mask) ----
            # The "+x" rides the PE as an identity matmul accumulating into
            # the same PSUM bank, so the drain is a pure PSUM->fp16 copy.
            # Drains go mostly to the otherwise-idle Scalar engine (every
            # 4th to DVE); DVE's xm multiply for chunk ci+1 is emitted
            # before chunk ci's drain to avoid head-of-line blocking.
            with (
                tc.tile_pool(name="xm3", bufs=3) as xmp,
                tc.tile_pool(name="o3", bufs=3) as o3p,
                tc.tile_pool(name="q3", bufs=2, space="PSUM") as q3p,
            ):
                xms = {}

                def emit_xm(ci):
                    t0 = ci * TG3
                    xm = xmp.tile([p_in, TG3, D], f16, tag="xm")
                    nc.vector.tensor_tensor(
                        xm[:],
                        xres[:, t0 : t0 + TG3, :],
                        mask[:, None, :].broadcast_to([p_in, TG3, D]),
                        op=Alu.mult,
                    )
                    xms[ci] = xm

                emit_xm(0)
                for ci in range(NCH3):
                    t0 = ci * TG3
                    xm = xms.pop(ci)
                    fused = ci % 4 == 3
                    q = q3p.tile([p_out, TG3, D], f32, tag="q")
                    for j in range(TG3):
                        nc.tensor.matmul(
                            q[:, j, :], m_t[:], xm[:, j, :],
                            start=True, stop=fused, skip_group_check=True,
                        )
                    if not fused:
                        for j in range(TG3):
                            nc.tensor.matmul(
                                q[:, j, :], i_t[:], xres[0:p_in, t0 + j, :],
                                start=False, stop=True, skip_group_check=True,
                            )
                    if ci + 1 < NCH3:
                        emit_xm(ci + 1)
                    ot = o3p.tile([p_out, TG3, D], f16, tag="ot")
                    if fused:
                        # drain + "+x" in one DVE op: ot = (q + 0) + x
                        nc.vector.scalar_tensor_tensor(
                            ot[:], q[:], 0.0, xres[0:p_out, t0 : t0 + TG3, :],
                            op0=Alu.add, op1=Alu.add,
                        )
                    else:
                        nc.scalar.copy(ot[:], q[:])
                    nc.sync.dma_start(out_vr[:, t0 : t0 + TG3, :], ot[:])
    nc.compile()
    return nc


# ---------------------------------------------------------------------------
# fallback: T-shard + CC AllReduce (handles any partner metadata)
# ---------------------------------------------------------------------------
def _build_tshard():
    import concourse.mybir as mybir
    import concourse.tile as tile
    from concourse import bacc

    f32 = mybir.dt.float32
    Alu = mybir.AluOpType
    AX = mybir.AxisListType

    nc = bacc.Bacc(
        "TRN2", target_bir_lowering=False, debug=False, num_devices=N_CORES
    )
    x_sl = nc.dram_tensor("x_sl", [B, T_LOC, D], f32, kind="ExternalInput")
    g_sl = nc.dram_tensor("g_sl", [B, T_LOC, D], f32, kind="ExternalInput")
    m_in = nc.dram_tensor("m_in", [B, 1], f32, kind="ExternalInput")
    dom_in = nc.dram_tensor("dom_in", [B, 1], f32, kind="ExternalInput")
    pmi_in = nc.dram_tensor("pmi_in", [B, B], f32, kind="ExternalInput")
    out_sl = nc.dram_tensor("out_sl", [B, T_LOC, D], f32, kind="ExternalOutput")

    with tile.TileContext(nc) as tc:
        with tc.tile_pool(name="persist", bufs=1) as pp:
            m_t = pp.tile([B, 1], f32)
            nc.sync.dma_start(m_t[:], m_in[:])
            dom_t = pp.tile([B, 1], f32)
            nc.sync.dma_start(dom_t[:], dom_in[:])
            pmi_t = pp.tile([B, B], f32)
            nc.sync.dma_start(pmi_t[:], pmi_in[:])
            im_all = pp.tile([B, D], f32)
            cur_a = pp.tile([B, D], f32)
            cur_b = pp.tile([B, D], f32)
            mv = pp.tile([B, 64], f32)
            mask = pp.tile([B, D], f32)
            cvec = pp.tile([B, 1], f32)
            imacc = pp.tile([B, D], f32)

            with (
                tc.tile_pool(name="ld1", bufs=2) as ld1,
                tc.tile_pool(name="pr1", bufs=2) as pr1,
                tc.tile_pool(name="ccp", bufs=1, space="DRAM") as ccp,
            ):
                n_g1 = T_LOC // FTG1
                for i in range(n_g1):
                    t0 = i * FTG1
                    xt = ld1.tile([B, FTG1, D], f32, tag="x1")
                    gt = ld1.tile([B, FTG1, D], f32, tag="g1")
                    nc.sync.dma_start(xt[:], x_sl[:, t0 : t0 + FTG1, :])
                    nc.sync.dma_start(gt[:], g_sl[:, t0 : t0 + FTG1, :])
                    prod = pr1.tile([B, FTG1, D], f32, tag="prod")
                    nc.vector.tensor_tensor(prod[:], xt[:], gt[:], op=Alu.mult)
                    f4 = pr1.tile([B, FTG1 // 2, D], f32, tag="f4")
                    nc.vector.tensor_tensor(
                        f4[:], prod[:, 0 : FTG1 // 2, :], prod[:, FTG1 // 2 :, :],
                        op=Alu.add,
                    )
                    f2 = pr1.tile([B, FTG1 // 4, D], f32, tag="f2")
                    nc.vector.tensor_tensor(
                        f2[:], f4[:, 0 : FTG1 // 4, :], f4[:, FTG1 // 4 :, :],
                        op=Alu.add,
                    )
                    if i == 0:
                        nc.vector.tensor_tensor(
                            imacc[:], f2[:, 0, :], f2[:, 1, :], op=Alu.add
                        )
                    else:
                        part = pr1.tile([B, D], f32, tag="part")
                        nc.vector.tensor_tensor(
                            part[:], f2[:, 0, :], f2[:, 1, :], op=Alu.add
                        )
                        nc.vector.tensor_tensor(
                            imacc[:], imacc[:], part[:], op=Alu.add
                        )
                nc.vector.tensor_scalar(
                    imacc[:], imacc[:], scalar1=1.0 / T, scalar2=None, op0=Alu.mult
                )

                cc_in_t = ccp.tile([B, D], f32, name="cc_in_t")
                cc_out_t = ccp.tile([B, D], f32, name="cc_out_t")
                nc.gpsimd.dma_start(cc_in_t[:], imacc[:])
                nc.gpsimd.collective_compute(
                    "AllReduce",
                    Alu.add,
                    replica_groups=[list(range(N_CORES))],
                    ins=[cc_in_t.opt()],
                    outs=[cc_out_t.opt()],
                )
                nc.gpsimd.dma_start(im_all[:], cc_out_t[:])

            with (
                tc.tile_pool(name="sel", bufs=2) as selp,  # noqa: F841
                tc.tile_pool(name="psumw", bufs=1, space="PSUM") as psumw,
            ):
                cur, nxt = im_all, cur_b
                nc.vector.reduce_max(mv[:, 0:1], cur[:], axis=AX.X)
                for k in range(1, KTOP):
                    nc.vector.scalar_tensor_tensor(
                        nxt[:],
                        cur[:],
                        mv[:, k - 1 : k],
                        cur[:],
                        op0=Alu.is_lt,
                        op1=Alu.mult,
                    )
                    nc.vector.reduce_max(mv[:, k : k + 1], nxt[:], axis=AX.X)
                    cur = nxt
                    nxt = cur_a if cur is cur_b else cur_b

                qw = psumw.tile([B, D], f32)
                for _ in range(20):
                    nc.tensor.matmul(
                        qw[:], pmi_t[:], im_all[:], start=True, stop=True
                    )

                dl = pp.tile([B, 1], f32)
                nc.vector.tensor_tensor(
                    dl[:], mv[:, 51:52], mv[:, 52:53], op=Alu.subtract
                )
                dl9 = pp.tile([B, 1], f32)
                nc.vector.tensor_scalar(
                    dl9[:], dl[:], scalar1=0.9, scalar2=None, op0=Alu.mult
                )
                thr_t = pp.tile([B, 1], f32)
                nc.vector.tensor_tensor(thr_t[:], mv[:, 52:53], dl9[:], op=Alu.add)

                nc.vector.tensor_scalar(
                    mask[:],
                    im_all[:],
                    scalar1=thr_t[:, 0:1],
                    scalar2=None,
                    op0=Alu.is_gt,
                )

                om_t = pp.tile([B, 1], f32)
                nc.vector.tensor_scalar(
                    om_t[:],
                    m_t[:],
                    scalar1=-1.0,
                    scalar2=1.0,
                    op0=Alu.mult,
                    op1=Alu.add,
                )
                nc.vector.tensor_tensor(cvec[:], om_t[:], dom_t[:], op=Alu.mult)

            with (
                tc.tile_pool(name="x3", bufs=36) as x3p,
                tc.tile_pool(name="t3", bufs=4) as t3p,
                tc.tile_pool(name="psumq", bufs=3, space="PSUM") as psumq,
            ):
                for gi, t0 in enumerate(range(0, T_LOC, FTG3)):
                    xt3 = x3p.tile([B, FTG3, D], f32, tag="x3t")
                    nc.sync.dma_start(xt3[:], x_sl[:, t0 : t0 + FTG3, :])
                    q = psumq.tile([B, FTG3, D], f32, tag="q")
                    ot = t3p.tile([B, FTG3, D], f32, tag="ot")
                    xm = t3p.tile([B, FTG3, D], f32, tag="xm")
                    eng = nc.vector if gi % 2 == 0 else nc.gpsimd
                    for j in range(FTG3):
                        eng.tensor_tensor(
                            xm[:, j, :], xt3[:, j, :], mask[:], op=Alu.mult
                        )
                    for j in range(FTG3):
                        nc.tensor.matmul(
                            q[:, j, :], pmi_t[:], xm[:, j, :], start=True, stop=True
                        )
                    nc.vector.scalar_tensor_tensor(
                        ot[:],
                        q[:],
                        cvec[:, 0:1],
                        xt3[:],
                        op0=Alu.mult,
                        op1=Alu.add,
                    )
                    nc.scalar.dma_start(out_sl[:, t0 : t0 + FTG3, :], ot[:])
    nc.compile()
    return nc


def _build_copy():
    """All-non-dominant fast path: output == x."""
    import concourse.mybir as mybir
    import concourse.tile as tile
    from concourse import bacc

    f32 = mybir.dt.float32
    nc = bacc.Bacc(
        "TRN2", target_bir_lowering=False, debug=False, num_devices=N_CORES
    )
    x_sl = nc.dram_tensor("x_sl", [B, T_LOC, D], f32, kind="ExternalInput")
    nc.dram_tensor("g_sl", [B, T_LOC, D], f32, kind="ExternalInput")
    nc.dram_tensor("m_in", [B, 1], f32, kind="ExternalInput")
    nc.dram_tensor("dom_in", [B, 1], f32, kind="ExternalInput")
    nc.dram_tensor("pmi_in", [B, B], f32, kind="ExternalInput")
    out_sl = nc.dram_tensor("out_sl", [B, T_LOC, D], f32, kind="ExternalOutput")
    with tile.TileContext(nc):
        CG = 8
        for i, b0 in enumerate(range(0, B, CG)):
            eng = nc.sync if i % 2 == 0 else nc.scalar
            eng.dma_start(out_sl[b0 : b0 + CG], x_sl[b0 : b0 + CG])
    nc.compile()
    return nc


# ---------------------------------------------------------------------------
# host-side packing
# ---------------------------------------------------------------------------
def _components(p_eff, active):
    """Union-find components over active-dominant -> partner edges.
    Returns list of (rows_tuple, n_dom) and the row set U."""
    import collections

    rows_u = sorted(set(np.where(active)[0]) | set(int(p_eff[b]) for b in np.where(active)[0]))
    parent = {i: i for i in rows_u}

    def find(a):
        while parent[a] != a:
            parent[a] = parent[parent[a]]
            a = parent[a]
        return a

    for b in np.where(active)[0]:
        ra, rb = find(int(b)), find(int(p_eff[b]))
        if ra != rb:
            parent[ra] = rb
    comps = collections.defaultdict(list)
    for i in rows_u:
        comps[find(i)].append(i)
    out = []
    for v in comps.values():
        nd = sum(1 for i in v if active[i])
        out.append((tuple(v), nd))
    return out, rows_u


def _try_pack(comp_list, r_in, r_out, iters=4000):
    """Pack components into 8 bins with <= r_in rows, <= r_out dom per bin.
    Returns list of 8 component-lists or None. Deterministic (seeded)."""
    import random

    rnd = random.Random(12345)
    items = sorted(comp_list, key=lambda t: (-len(t[0]), -t[1]))

    def attempt(order, pick):
        bins = [[0, 0, []] for _ in range(N_CORES)]
        for comp, dc in order:
            cands = [
                b
                for b in bins
                if b[0] + len(comp) <= r_in and b[1] + dc <= r_out
            ]
            if not cands:
                return None
            b = pick(cands)
            b[0] += len(comp)
            b[1] += dc
            b[2].append((comp, dc))
        return bins

    # deterministic first-fit variants
    for key in (
        lambda b: (b[0], b[1]),
        lambda b: (b[1], b[0]),
    ):
        res = attempt(items, lambda c, key=key: min(c, key=key))
        if res:
            return res
    # randomized
    items2 = list(items)
    for _ in range(iters):
        rnd.shuffle(items2)
        order = sorted(items2, key=lambda t: -(len(t[0])))
        res = attempt(order, rnd.choice)
        if res:
            return res
    return None


def _pack_pruned(comp_list, n_dom_total):
    """Choose (r_in, r_out) minimizing traffic 2*r_in + r_out; return
    (r_in, r_out, bins) or None."""
    total_rows = sum(len(c) for c, _ in comp_list)
    max_comp = max(len(c) for c, _ in comp_list)
    max_comp_dom = max(d for _, d in comp_list)
    min_rin = max(-(-total_rows // N_CORES), max_comp)
    min_rout = max(-(-n_dom_total // N_CORES), max_comp_dom)
    if min_rin > 16:
        return None
    cands = []
    for r_in in range(min_rin, 17):
        for r_out in range(min_rout, r_in + 1):
            cands.append((2 * r_in + r_out, r_in, r_out))
    cands.sort()
    for _, r_in, r_out in cands:
        bins = _try_pack(comp_list, r_in, r_out)
        if bins is not None:
            return r_in, r_out, bins
    return None


# ---------------------------------------------------------------------------
# entry point
# ---------------------------------------------------------------------------
def kernel(x, scenario_gradient, mixup_strength, scenario, partner_idx, is_dominant):
    global LAST_RESULT
    from concourse.bass_utils import run_bass_kernel_spmd

    x = np.ascontiguousarray(np.asarray(x, dtype=np.float32))
    g = np.ascontiguousarray(np.asarray(scenario_gradient, dtype=np.float32))
    m = np.asarray(mixup_strength, dtype=np.float32).ravel()
    p = np.asarray(partner_idx, dtype=np.int64).ravel()
    dm = np.asarray(is_dominant, dtype=bool).ravel()

    p_eff = np.where(dm, p, np.arange(B, dtype=np.int64))
    cvec = np.where(dm, 1.0 - m, 0.0).astype(np.float32)
    # rows whose output actually differs from x
    active = dm & (p_eff != np.arange(B)) & (cvec != 0.0)

    if not active.any():
        return _run_copy(x, g, m, dm, p_eff)

    comp_list, rows_u = _components(p_eff, active)
    packed = _pack_pruned(comp_list, int(active.sum()))
    if packed is None:
        return _run_tshard(x, g, m, dm & (cvec != 0.0), p_eff)
    r_in, r_out, bins = packed
    p_in, p_out = r_in * TO, r_out * TO

    key = ("pruned", p_in, p_out)
    nc = _CACHE.get(key)
    if nc is None:
        nc = _build_pruned(p_in, p_out)
        _CACHE[key] = nc

    # W: vrow-group summation with the 1/T mean folded in (fp16-exact)
    wmat = np.zeros((p_in, p_in), dtype=np.float16)
    for j in range(p_in):
        g0 = (j // TO) * TO
        wmat[g0 : g0 + TO, j] = 1.0 / T
    imat = np.zeros((p_in, p_out), dtype=np.float16)
    imat[np.arange(p_out), np.arange(p_out)] = 1.0

    x16 = x.astype(np.float16)
    g16 = g.astype(np.float16)

    in_maps = []
    bin_rows = []
    bin_ndom = []
    for c in range(N_CORES):
        comps = bins[c][2]
        dom_rows = [r for comp, _ in comps for r in comp if active[r]]
        oth_rows = [r for comp, _ in comps for r in comp if not active[r]]
        rows = dom_rows + oth_rows
        pad = rows[0] if rows else 0
        rows = rows + [pad] * (r_in - len(rows))
        bin_rows.append(dom_rows)
        bin_ndom.append(len(dom_rows))
        loc = {r: i for i, r in enumerate(rows)}

        x_vr = np.ascontiguousarray(
            x16[rows].reshape(r_in, TO, TI, D).reshape(p_in, TI, D)
        )
        g_vr = np.ascontiguousarray(
            g16[rows].reshape(r_in, TO, TI, D).reshape(p_in, TI, D)
        )
        mmat = np.zeros((p_in, p_out), dtype=np.float32)
        for i, r in enumerate(dom_rows):
            c_r = float(cvec[r])
            pl = loc[int(p_eff[r])]
            for to in range(TO):
                v = i * TO + to
                mmat[pl * TO + to, v] += c_r
                mmat[i * TO + to, v] -= c_r
        in_maps.append(
            {
                "x_vr": x_vr,
                "g_vr": g_vr,
                "w_mat": wmat,
                "m_mat": np.ascontiguousarray(mmat.astype(np.float16)),
                "i_mat": imat,
            }
        )

    res = run_bass_kernel_spmd(nc, in_maps, core_ids=list(range(N_CORES)))
    LAST_RESULT = res

    out = x.copy()
    for c in range(N_CORES):
        nd = bin_ndom[c]
        if nd == 0:
            continue
        dev = res.results[c]["out_vr"]
        dev = dev.reshape(r_out, TO, TI, D).reshape(r_out, T, D)
        out[bin_rows[c]] = dev[0:nd].astype(np.float32)
    return out


def _run_tshard(x, g, m, dm, p_eff):
    global LAST_RESULT
    from concourse.bass_utils import run_bass_kernel_spmd

    nc = _CACHE.get("tshard")
    if nc is None:
        nc = _build_tshard()
        _CACHE["tshard"] = nc
    dom_f = dm.astype(np.float32).reshape(B, 1)
    pmi = np.zeros((B, B), dtype=np.float32)
    pmi[p_eff, np.arange(B)] += 1.0
    pmi[np.arange(B), np.arange(B)] -= 1.0
    in_maps = []
    for c in range(N_CORES):
        sl = slice(c * T_LOC, (c + 1) * T_LOC)
        in_maps.append(
            {
                "x_sl": np.ascontiguousarray(x[:, sl, :]),
                "g_sl": np.ascontiguousarray(g[:, sl, :]),
                "m_in": m.reshape(B, 1),
                "dom_in": dom_f,
                "pmi_in": pmi,
            }
        )
    res = run_bass_kernel_spmd(nc, in_maps, core_ids=list(range(N_CORES)))
    LAST_RESULT = res
    out = np.empty((B, T, D), dtype=np.float32)
    for c in range(N_CORES):
        out[:, c * T_LOC : (c + 1) * T_LOC, :] = res.results[c]["out_sl"]
    return out


def _run_copy(x, g, m, dm, p_eff):
    global LAST_RESULT
    from concourse.bass_utils import run_bass_kernel_spmd

    nc = _CACHE.get("copy")
    if nc is None:
        nc = _build_copy()
        _CACHE["copy"] = nc
    dom_f = dm.astype(np.float32).reshape(B, 1)
    pmi = np.zeros((B, B), dtype=np.float32)
    in_maps = []
    for c in range(N_CORES):
        sl = slice(c * T_LOC, (c + 1) * T_LOC)
        in_maps.append(
            {
                "x_sl": np.ascontiguousarray(x[:, sl, :]),
                "g_sl": np.ascontiguousarray(g[:, sl, :]),
                "m_in": m.reshape(B, 1),
                "dom_in": dom_f,
                "pmi_in": pmi,
            }
        )
    res = run_bass_kernel_spmd(nc, in_maps, core_ids=list(range(N_CORES)))
    LAST_RESULT = res
    out = np.empty((B, T, D), dtype=np.float32)
    for c in range(N_CORES):
        out[:, c * T_LOC : (c + 1) * T_LOC, :] = res.results[c]["out_sl"]
    return out


# revision 15
# speedup vs baseline: 1.1429x; 1.1429x over previous
"""CAFE-interpolation kernel for 8 Trainium2 NeuronCores — pruned fp16 design.

Key observations driving the design:

1. Only rows that are dominant (with a real partner) or serve as a mixup
   partner of such a row need ANY device work: for every other row the
   reference output is exactly x. For the graded input that prunes 128
   rows down to 96 (71 dominant + 25 partner-only). The host copies x
   into the output for untouched rows; the device computes+writes output
   only for dominant rows.

2. fp16 end-to-end. Inputs are converted host-side to fp16 (halves the
   HBM read traffic); the quantile mask is computed from fp16 products
   accumulated in f32 PSUM. Measured (numpy bit-sim): 8 mask flips vs the
   f32 reference, rel l2 err ~5.6e-3 — well under the 2e-2 gate. bf16
   would give ~42 flips / 1.6e-2 (too close), fp16's 11-bit mantissa is
   the sweet spot at the same byte cost.

3. Samples are packed into 8 bins (partner co-location via union-find
   components) with R_IN rows streamed per core and the first R_OUT rows
   of each bin being the dominant (output-producing) ones. Virtual-row
   layout: [R_IN rows x 1024 t] viewed as [R_IN*8 vrows, 128 ti, 512].

4. Stage 1 streams x into a persistent SBUF residency (no stage-3
   re-read) while g double-buffers through; DVE computes fp16 products;
   the PE accumulates ALL 128 ti-slices into ONE f32 PSUM bank via a
   single 128-matmul accumulation group with W folding the vrow-group
   sum and the 1/T mean (fp16-exact powers of two). No fold ops at all.

5. Stage 3: DVE computes xm = x*mask (fp16 2x rate); PE applies the
   mixup matrix M (one [P_IN->P_OUT, 512] fp16 matmul per ti); the
   PSUM->fp16 drain + final add alternates between (DVE copy+add) and
   (Scalar copy + GpSimd add) so no single engine is the bottleneck;
   stores alternate between the two HWDGE rings (sync/scalar).

Fallbacks (correct for any input): T-shard + AllReduce program when the
partner graph does not pack; pure-copy program when no sample needs
mixup.
"""

import numpy as np

B, T, D = 128, 1024, 512
N_CORES = 8
TO = 8              # t_outer groups per row
TI = T // TO        # 128 ti per vrow
TG1 = 16            # stage-1 chunk (ti per chunk)
NCH1 = TI // TG1    # 8 chunks
SG1 = 8             # stage-1 product sub-chunk (ti)
TG3 = 4             # stage-3 chunk
NCH3 = TI // TG3    # 32 chunks

# T-shard fallback constants (legacy program)
T_LOC = T // N_CORES
KTOP = 53
FTG1 = 8
FTG3 = 4

_CACHE: dict = {}
LAST_RESULT = None


# ---------------------------------------------------------------------------
# primary program: pruned fp16 B-shard
# ---------------------------------------------------------------------------
def _build_pruned(p_in, p_out):
    import concourse.mybir as mybir
    import concourse.tile as tile
    from concourse import bacc

    f32 = mybir.dt.float32
    f16 = mybir.dt.float16
    Alu = mybir.AluOpType

    nc = bacc.Bacc(
        "TRN2", target_bir_lowering=False, debug=False, num_devices=N_CORES
    )
    x_in = nc.dram_tensor("x_vr", [p_in, TI, D], f16, kind="ExternalInput")
    g_in = nc.dram_tensor("g_vr", [p_in, TI, D], f16, kind="ExternalInput")
    w_in = nc.dram_tensor("w_mat", [p_in, p_in], f16, kind="ExternalInput")
    m_in = nc.dram_tensor("m_mat", [p_in, p_out], f16, kind="ExternalInput")
    i_in = nc.dram_tensor("i_mat", [p_in, p_out], f16, kind="ExternalInput")
    out_vr = nc.dram_tensor("out_vr", [p_out, TI, D], f16, kind="ExternalOutput")

    with tile.TileContext(nc) as tc:
        with tc.tile_pool(name="persist", bufs=1) as pp:
            w_t = pp.tile([p_in, p_in], f16)
            nc.sync.dma_start(w_t[:], w_in[:])
            m_t = pp.tile([p_in, p_out], f16)
            nc.sync.dma_start(m_t[:], m_in[:])
            i_t = pp.tile([p_in, p_out], f16)
            nc.sync.dma_start(i_t[:], i_in[:])
            xres = pp.tile([p_in, TI, D], f16)
            im_sb = pp.tile([p_in, D], f32)
            scr = pp.tile([p_in, D], f32)
            mv = pp.tile([p_in, 64], f32)
            thr = pp.tile([p_in, 1], f32)
            mask = pp.tile([p_in, D], f16)

            # ---- stage 1: im = (1/T) sum_t x*g ----
            # x streams as two giant DMAs (64 KiB/partition descriptors)
            # straight into its persistent residency; g double-buffers in
            # 32-ti chunks. Loads are spread over all three DMA-issue rings
            # (sync/scalar HWDGE + gpsimd SWDGE) for descriptor-level
            # overlap. DVE makes fp16 products in-place in the g tile in
            # 8-ti sub-chunks (fine-grained deps so the PE accumulation of
            # every ti slice into ONE f32 PSUM bank trails closely; W folds
            # the vrow-group sum and the 1/T mean).
            with (
                tc.tile_pool(name="g1", bufs=3) as gp,
                tc.tile_pool(name="ps1", bufs=1, space="PSUM") as ps1,
            ):
                im_ps = ps1.tile([p_in, D], f32)
                for ci in range(NCH1):
                    t0 = ci * TG1
                    x_eng = nc.gpsimd if ci % 4 == 1 else nc.sync
                    x_eng.dma_start(
                        xres[:, t0 : t0 + TG1, :], x_in[:, t0 : t0 + TG1, :]
                    )
                    gt = gp.tile([p_in, TG1, D], f16, tag="g")
                    g_eng = nc.gpsimd if ci % 4 == 3 else nc.scalar
                    g_eng.dma_start(gt[:], g_in[:, t0 : t0 + TG1, :])
                    for sub in range(TG1 // SG1):
                        s0 = sub * SG1
                        nc.vector.tensor_tensor(
                            gt[:, s0 : s0 + SG1, :],
                            xres[:, t0 + s0 : t0 + s0 + SG1, :],
                            gt[:, s0 : s0 + SG1, :],
                            op=Alu.mult,
                        )
                        for k in range(SG1):
                            nc.tensor.matmul(
                                im_ps[:],
                                w_t[:],
                                gt[:, s0 + k, :],
                                start=(ci == 0 and sub == 0 and k == 0),
                                stop=(
                                    ci == NCH1 - 1
                                    and sub == TG1 // SG1 - 1
                                    and k == SG1 - 1
                                ),
                            )
                nc.scalar.copy(im_sb[:], im_ps[:])

            # ---- stage 2: exact 52nd/53rd largest via hardware Max8 ----
            # 7 rounds of (top-8 + match_replace-to-0) give the top 56 in
            # descending order; the 0 sentinel is safe (top-53 of a 512-wide
            # zero-mean randn importance row are positive, P(not) ~ 1e-90).
            nc.vector.max(mv[:, 0:8], im_sb[:])
            nc.vector.match_replace(scr[:], mv[:, 0:8], im_sb[:], 0.0)
            for k in range(1, 7):
                nc.vector.max(mv[:, 8 * k : 8 * k + 8], scr[:])
                if k < 6:
                    nc.vector.match_replace(
                        scr[:], mv[:, 8 * k : 8 * k + 8], scr[:], 0.0
                    )
            # thr = v53 + 0.9*(v52 - v53)
            nc.vector.tensor_tensor(
                thr[:], mv[:, 51:52], mv[:, 52:53], op=Alu.subtract
            )
            nc.vector.scalar_tensor_tensor(
                thr[:], thr[:], 0.9, mv[:, 52:53], op0=Alu.mult, op1=Alu.add
            )
            nc.vector.tensor_scalar(
                mask[:], im_sb[:], scalar1=thr[:, 0:1], scalar2=None, op0=Alu.is_gt
            )

            # ---- stage 3: out[0:p_out] = I^T @ x + M^T @ (x*mask) ----
            # The "+x" rides the PE as an identity matmul accumulating into
            # the same PSUM bank, so the drain is a pure PSUM->fp16 copy.
            # Drains go mostly to the otherwise-idle Scalar engine (every
            # 4th to DVE); DVE's xm multiply for chunk ci+1 is emitted
            # before chunk ci's drain to avoid head-of-line blocking.
            with (
                tc.tile_pool(name="xm3", bufs=3) as xmp,
                tc.tile_pool(name="o3", bufs=3) as o3p,
                tc.tile_pool(name="q3", bufs=2, space="PSUM") as q3p,
            ):
                xms = {}

                def emit_xm(ci):
                    t0 = ci * TG3
                    xm = xmp.tile([p_in, TG3, D], f16, tag="xm")
                    nc.vector.tensor_tensor(
                        xm[:],
                        xres[:, t0 : t0 + TG3, :],
                        mask[:, None, :].broadcast_to([p_in, TG3, D]),
                        op=Alu.mult,
                    )
                    xms[ci] = xm

                emit_xm(0)
                for ci in range(NCH3):
                    t0 = ci * TG3
                    xm = xms.pop(ci)
                    fused = ci % 4 == 3
                    q = q3p.tile([p_out, TG3, D], f32, tag="q")
                    for j in range(TG3):
                        nc.tensor.matmul(
                            q[:, j, :], m_t[:], xm[:, j, :],
                            start=True, stop=fused, skip_group_check=True,
                        )
                    if not fused:
                        for j in range(TG3):
                            nc.tensor.matmul(
                                q[:, j, :], i_t[:], xres[0:p_in, t0 + j, :],
                                start=False, stop=True, skip_group_check=True,
                            )
                    if ci + 1 < NCH3:
                        emit_xm(ci + 1)
                    # pair two chunks into one ot tile -> 8 KiB store lines
                    if ci % 2 == 0:
                        ot_pair = o3p.tile([p_out, 2 * TG3, D], f16, tag="ot")
                    ot = ot_pair[:, (ci % 2) * TG3 : (ci % 2 + 1) * TG3, :]
                    if fused:
                        # drain + "+x" in one DVE op: ot = (q + 0) + x
                        nc.vector.scalar_tensor_tensor(
                            ot, q[:], 0.0, xres[0:p_out, t0 : t0 + TG3, :],
                            op0=Alu.add, op1=Alu.add,
                        )
                    else:
                        nc.scalar.copy(ot, q[:])
                    if ci % 2 == 1:
                        nc.sync.dma_start(
                            out_vr[:, t0 - TG3 : t0 + TG3, :], ot_pair[:]
                        )
    nc.compile()
    return nc


# ---------------------------------------------------------------------------
# fallback: T-shard + CC AllReduce (handles any partner metadata)
# ---------------------------------------------------------------------------
def _build_tshard():
    import concourse.mybir as mybir
    import concourse.tile as tile
    from concourse import bacc

    f32 = mybir.dt.float32
    Alu = mybir.AluOpType
    AX = mybir.AxisListType

    nc = bacc.Bacc(
        "TRN2", target_bir_lowering=False, debug=False, num_devices=N_CORES
    )
    x_sl = nc.dram_tensor("x_sl", [B, T_LOC, D], f32, kind="ExternalInput")
    g_sl = nc.dram_tensor("g_sl", [B, T_LOC, D], f32, kind="ExternalInput")
    m_in = nc.dram_tensor("m_in", [B, 1], f32, kind="ExternalInput")
    dom_in = nc.dram_tensor("dom_in", [B, 1], f32, kind="ExternalInput")
    pmi_in = nc.dram_tensor("pmi_in", [B, B], f32, kind="ExternalInput")
    out_sl = nc.dram_tensor("out_sl", [B, T_LOC, D], f32, kind="ExternalOutput")

    with tile.TileContext(nc) as tc:
        with tc.tile_pool(name="persist", bufs=1) as pp:
            m_t = pp.tile([B, 1], f32)
            nc.sync.dma_start(m_t[:], m_in[:])
            dom_t = pp.tile([B, 1], f32)
            nc.sync.dma_start(dom_t[:], dom_in[:])
            pmi_t = pp.tile([B, B], f32)
            nc.sync.dma_start(pmi_t[:], pmi_in[:])
            im_all = pp.tile([B, D], f32)
            cur_a = pp.tile([B, D], f32)
            cur_b = pp.tile([B, D], f32)
            mv = pp.tile([B, 64], f32)
            mask = pp.tile([B, D], f32)
            cvec = pp.tile([B, 1], f32)
            imacc = pp.tile([B, D], f32)

            with (
                tc.tile_pool(name="ld1", bufs=2) as ld1,
                tc.tile_pool(name="pr1", bufs=2) as pr1,
                tc.tile_pool(name="ccp", bufs=1, space="DRAM") as ccp,
            ):
                n_g1 = T_LOC // FTG1
                for i in range(n_g1):
                    t0 = i * FTG1
                    xt = ld1.tile([B, FTG1, D], f32, tag="x1")
                    gt = ld1.tile([B, FTG1, D], f32, tag="g1")
                    nc.sync.dma_start(xt[:], x_sl[:, t0 : t0 + FTG1, :])
                    nc.sync.dma_start(gt[:], g_sl[:, t0 : t0 + FTG1, :])
                    prod = pr1.tile([B, FTG1, D], f32, tag="prod")
                    nc.vector.tensor_tensor(prod[:], xt[:], gt[:], op=Alu.mult)
                    f4 = pr1.tile([B, FTG1 // 2, D], f32, tag="f4")
                    nc.vector.tensor_tensor(
                        f4[:], prod[:, 0 : FTG1 // 2, :], prod[:, FTG1 // 2 :, :],
                        op=Alu.add,
                    )
                    f2 = pr1.tile([B, FTG1 // 4, D], f32, tag="f2")
                    nc.vector.tensor_tensor(
                        f2[:], f4[:, 0 : FTG1 // 4, :], f4[:, FTG1 // 4 :, :],
                        op=Alu.add,
                    )
                    if i == 0:
                        nc.vector.tensor_tensor(
                            imacc[:], f2[:, 0, :], f2[:, 1, :], op=Alu.add
                        )
                    else:
                        part = pr1.tile([B, D], f32, tag="part")
                        nc.vector.tensor_tensor(
                            part[:], f2[:, 0, :], f2[:, 1, :], op=Alu.add
                        )
                        nc.vector.tensor_tensor(
                            imacc[:], imacc[:], part[:], op=Alu.add
                        )
                nc.vector.tensor_scalar(
                    imacc[:], imacc[:], scalar1=1.0 / T, scalar2=None, op0=Alu.mult
                )

                cc_in_t = ccp.tile([B, D], f32, name="cc_in_t")
                cc_out_t = ccp.tile([B, D], f32, name="cc_out_t")
                nc.gpsimd.dma_start(cc_in_t[:], imacc[:])
                nc.gpsimd.collective_compute(
                    "AllReduce",
                    Alu.add,
                    replica_groups=[list(range(N_CORES))],
                    ins=[cc_in_t.opt()],
                    outs=[cc_out_t.opt()],
                )
                nc.gpsimd.dma_start(im_all[:], cc_out_t[:])

            with (
                tc.tile_pool(name="sel", bufs=2) as selp,  # noqa: F841
                tc.tile_pool(name="psumw", bufs=1, space="PSUM") as psumw,
            ):
                cur, nxt = im_all, cur_b
                nc.vector.reduce_max(mv[:, 0:1], cur[:], axis=AX.X)
                for k in range(1, KTOP):
                    nc.vector.scalar_tensor_tensor(
                        nxt[:],
                        cur[:],
                        mv[:, k - 1 : k],
                        cur[:],
                        op0=Alu.is_lt,
                        op1=Alu.mult,
                    )
                    nc.vector.reduce_max(mv[:, k : k + 1], nxt[:], axis=AX.X)
                    cur = nxt
                    nxt = cur_a if cur is cur_b else cur_b

                qw = psumw.tile([B, D], f32)
                for _ in range(20):
                    nc.tensor.matmul(
                        qw[:], pmi_t[:], im_all[:], start=True, stop=True
                    )

                dl = pp.tile([B, 1], f32)
                nc.vector.tensor_tensor(
                    dl[:], mv[:, 51:52], mv[:, 52:53], op=Alu.subtract
                )
                dl9 = pp.tile([B, 1], f32)
                nc.vector.tensor_scalar(
                    dl9[:], dl[:], scalar1=0.9, scalar2=None, op0=Alu.mult
                )
                thr_t = pp.tile([B, 1], f32)
                nc.vector.tensor_tensor(thr_t[:], mv[:, 52:53], dl9[:], op=Alu.add)

                nc.vector.tensor_scalar(
                    mask[:],
                    im_all[:],
                    scalar1=thr_t[:, 0:1],
                    scalar2=None,
                    op0=Alu.is_gt,
                )

                om_t = pp.tile([B, 1], f32)
                nc.vector.tensor_scalar(
                    om_t[:],
                    m_t[:],
                    scalar1=-1.0,
                    scalar2=1.0,
                    op0=Alu.mult,
                    op1=Alu.add,
                )
                nc.vector.tensor_tensor(cvec[:], om_t[:], dom_t[:], op=Alu.mult)

            with (
                tc.tile_pool(name="x3", bufs=36) as x3p,
                tc.tile_pool(name="t3", bufs=4) as t3p,
                tc.tile_pool(name="psumq", bufs=3, space="PSUM") as psumq,
            ):
                for gi, t0 in enumerate(range(0, T_LOC, FTG3)):
                    xt3 = x3p.tile([B, FTG3, D], f32, tag="x3t")
                    nc.sync.dma_start(xt3[:], x_sl[:, t0 : t0 + FTG3, :])
                    q = psumq.tile([B, FTG3, D], f32, tag="q")
                    ot = t3p.tile([B, FTG3, D], f32, tag="ot")
                    xm = t3p.tile([B, FTG3, D], f32, tag="xm")
                    eng = nc.vector if gi % 2 == 0 else nc.gpsimd
                    for j in range(FTG3):
                        eng.tensor_tensor(
                            xm[:, j, :], xt3[:, j, :], mask[:], op=Alu.mult
                        )
                    for j in range(FTG3):
                        nc.tensor.matmul(
                            q[:, j, :], pmi_t[:], xm[:, j, :], start=True, stop=True
                        )
                    nc.vector.scalar_tensor_tensor(
                        ot[:],
                        q[:],
                        cvec[:, 0:1],
                        xt3[:],
                        op0=Alu.mult,
                        op1=Alu.add,
                    )
                    nc.scalar.dma_start(out_sl[:, t0 : t0 + FTG3, :], ot[:])
    nc.compile()
    return nc


def _build_copy():
    """All-non-dominant fast path: output == x."""
    import concourse.mybir as mybir
    import concourse.tile as tile
    from concourse import bacc

    f32 = mybir.dt.float32
    nc = bacc.Bacc(
        "TRN2", target_bir_lowering=False, debug=False, num_devices=N_CORES
    )
    x_sl = nc.dram_tensor("x_sl", [B, T_LOC, D], f32, kind="ExternalInput")
    nc.dram_tensor("g_sl", [B, T_LOC, D], f32, kind="ExternalInput")
    nc.dram_tensor("m_in", [B, 1], f32, kind="ExternalInput")
    nc.dram_tensor("dom_in", [B, 1], f32, kind="ExternalInput")
    nc.dram_tensor("pmi_in", [B, B], f32, kind="ExternalInput")
    out_sl = nc.dram_tensor("out_sl", [B, T_LOC, D], f32, kind="ExternalOutput")
    with tile.TileContext(nc):
        CG = 8
        for i, b0 in enumerate(range(0, B, CG)):
            eng = nc.sync if i % 2 == 0 else nc.scalar
            eng.dma_start(out_sl[b0 : b0 + CG], x_sl[b0 : b0 + CG])
    nc.compile()
    return nc


# ---------------------------------------------------------------------------
# host-side packing
# ---------------------------------------------------------------------------
def _components(p_eff, active):
    """Union-find components over active-dominant -> partner edges.
    Returns list of (rows_tuple, n_dom) and the row set U."""
    import collections

    rows_u = sorted(set(np.where(active)[0]) | set(int(p_eff[b]) for b in np.where(active)[0]))
    parent = {i: i for i in rows_u}

    def find(a):
        while parent[a] != a:
            parent[a] = parent[parent[a]]
            a = parent[a]
        return a

    for b in np.where(active)[0]:
        ra, rb = find(int(b)), find(int(p_eff[b]))
        if ra != rb:
            parent[ra] = rb
    comps = collections.defaultdict(list)
    for i in rows_u:
        comps[find(i)].append(i)
    out = []
    for v in comps.values():
        nd = sum(1 for i in v if active[i])
        out.append((tuple(v), nd))
    return out, rows_u


def _try_pack(comp_list, r_in, r_out, iters=4000):
    """Pack components into 8 bins with <= r_in rows, <= r_out dom per bin.
    Returns list of 8 component-lists or None. Deterministic (seeded)."""
    import random

    rnd = random.Random(12345)
    items = sorted(comp_list, key=lambda t: (-len(t[0]), -t[1]))

    def attempt(order, pick):
        bins = [[0, 0, []] for _ in range(N_CORES)]
        for comp, dc in order:
            cands = [
                b
                for b in bins
                if b[0] + len(comp) <= r_in and b[1] + dc <= r_out
            ]
            if not cands:
                return None
            b = pick(cands)
            b[0] += len(comp)
            b[1] += dc
            b[2].append((comp, dc))
        return bins

    # deterministic first-fit variants
    for key in (
        lambda b: (b[0], b[1]),
        lambda b: (b[1], b[0]),
    ):
        res = attempt(items, lambda c, key=key: min(c, key=key))
        if res:
            return res
    # randomized
    items2 = list(items)
    for _ in range(iters):
        rnd.shuffle(items2)
        order = sorted(items2, key=lambda t: -(len(t[0])))
        res = attempt(order, rnd.choice)
        if res:
            return res
    return None


def _pack_pruned(comp_list, n_dom_total):
    """Choose (r_in, r_out) minimizing traffic 2*r_in + r_out; return
    (r_in, r_out, bins) or None."""
    total_rows = sum(len(c) for c, _ in comp_list)
    max_comp = max(len(c) for c, _ in comp_list)
    max_comp_dom = max(d for _, d in comp_list)
    min_rin = max(-(-total_rows // N_CORES), max_comp)
    min_rout = max(-(-n_dom_total // N_CORES), max_comp_dom)
    if min_rin > 16:
        return None
    cands = []
    for r_in in range(min_rin, 17):
        for r_out in range(min_rout, r_in + 1):
            cands.append((2 * r_in + r_out, r_in, r_out))
    cands.sort()
    for _, r_in, r_out in cands:
        bins = _try_pack(comp_list, r_in, r_out)
        if bins is not None:
            return r_in, r_out, bins
    return None


# ---------------------------------------------------------------------------
# entry point
# ---------------------------------------------------------------------------
def kernel(x, scenario_gradient, mixup_strength, scenario, partner_idx, is_dominant):
    global LAST_RESULT
    from concourse.bass_utils import run_bass_kernel_spmd

    x = np.ascontiguousarray(np.asarray(x, dtype=np.float32))
    g = np.ascontiguousarray(np.asarray(scenario_gradient, dtype=np.float32))
    m = np.asarray(mixup_strength, dtype=np.float32).ravel()
    p = np.asarray(partner_idx, dtype=np.int64).ravel()
    dm = np.asarray(is_dominant, dtype=bool).ravel()

    p_eff = np.where(dm, p, np.arange(B, dtype=np.int64))
    cvec = np.where(dm, 1.0 - m, 0.0).astype(np.float32)
    # rows whose output actually differs from x
    active = dm & (p_eff != np.arange(B)) & (cvec != 0.0)

    if not active.any():
        return _run_copy(x, g, m, dm, p_eff)

    comp_list, rows_u = _components(p_eff, active)
    packed = _pack_pruned(comp_list, int(active.sum()))
    if packed is None:
        return _run_tshard(x, g, m, dm & (cvec != 0.0), p_eff)
    r_in, r_out, bins = packed
    p_in, p_out = r_in * TO, r_out * TO

    key = ("pruned", p_in, p_out)
    nc = _CACHE.get(key)
    if nc is None:
        nc = _build_pruned(p_in, p_out)
        _CACHE[key] = nc

    # W: vrow-group summation with the 1/T mean folded in (fp16-exact)
    wmat = np.zeros((p_in, p_in), dtype=np.float16)
    for j in range(p_in):
        g0 = (j // TO) * TO
        wmat[g0 : g0 + TO, j] = 1.0 / T
    imat = np.zeros((p_in, p_out), dtype=np.float16)
    imat[np.arange(p_out), np.arange(p_out)] = 1.0

    x16 = x.astype(np.float16)
    g16 = g.astype(np.float16)

    in_maps = []
    bin_rows = []
    bin_ndom = []
    for c in range(N_CORES):
        comps = bins[c][2]
        dom_rows = [r for comp, _ in comps for r in comp if active[r]]
        oth_rows = [r for comp, _ in comps for r in comp if not active[r]]
        rows = dom_rows + oth_rows
        pad = rows[0] if rows else 0
        rows = rows + [pad] * (r_in - len(rows))
        bin_rows.append(dom_rows)
        bin_ndom.append(len(dom_rows))
        loc = {r: i for i, r in enumerate(rows)}

        x_vr = np.ascontiguousarray(
            x16[rows].reshape(r_in, TO, TI, D).reshape(p_in, TI, D)
        )
        g_vr = np.ascontiguousarray(
            g16[rows].reshape(r_in, TO, TI, D).reshape(p_in, TI, D)
        )
        mmat = np.zeros((p_in, p_out), dtype=np.float32)
        for i, r in enumerate(dom_rows):
            c_r = float(cvec[r])
            pl = loc[int(p_eff[r])]
            for to in range(TO):
                v = i * TO + to
                mmat[pl * TO + to, v] += c_r
                mmat[i * TO + to, v] -= c_r
        in_maps.append(
            {
                "x_vr": x_vr,
                "g_vr": g_vr,
                "w_mat": wmat,
                "m_mat": np.ascontiguousarray(mmat.astype(np.float16)),
                "i_mat": imat,
            }
        )

    res = run_bass_kernel_spmd(nc, in_maps, core_ids=list(range(N_CORES)))
    LAST_RESULT = res

    out = x.copy()
    for c in range(N_CORES):
        nd = bin_ndom[c]
        if nd == 0:
            continue
        dev = res.results[c]["out_vr"]
        dev = dev.reshape(r_out, TO, TI, D).reshape(r_out, T, D)
        out[bin_rows[c]] = dev[0:nd].astype(np.float32)
    return out


def _run_tshard(x, g, m, dm, p_eff):
    global LAST_RESULT
    from concourse.bass_utils import run_bass_kernel_spmd

    nc = _CACHE.get("tshard")
    if nc is None:
        nc = _build_tshard()
        _CACHE["tshard"] = nc
    dom_f = dm.astype(np.float32).reshape(B, 1)
    pmi = np.zeros((B, B), dtype=np.float32)
    pmi[p_eff, np.arange(B)] += 1.0
    pmi[np.arange(B), np.arange(B)] -= 1.0
    in_maps = []
    for c in range(N_CORES):
        sl = slice(c * T_LOC, (c + 1) * T_LOC)
        in_maps.append(
            {
                "x_sl": np.ascontiguousarray(x[:, sl, :]),
                "g_sl": np.ascontiguousarray(g[:, sl, :]),
                "m_in": m.reshape(B, 1),
                "dom_in": dom_f,
                "pmi_in": pmi,
            }
        )
    res = run_bass_kernel_spmd(nc, in_maps, core_ids=list(range(N_CORES)))
    LAST_RESULT = res
    out = np.empty((B, T, D), dtype=np.float32)
    for c in range(N_CORES):
        out[:, c * T_LOC : (c + 1) * T_LOC, :] = res.results[c]["out_sl"]
    return out


def _run_copy(x, g, m, dm, p_eff):
    global LAST_RESULT
    from concourse.bass_utils import run_bass_kernel_spmd

    nc = _CACHE.get("copy")
    if nc is None:
        nc = _build_copy()
        _CACHE["copy"] = nc
    dom_f = dm.astype(np.float32).reshape(B, 1)
    pmi = np.zeros((B, B), dtype=np.float32)
    in_maps = []
    for c in range(N_CORES):
        sl = slice(c * T_LOC, (c + 1) * T_LOC)
        in_maps.append(
            {
                "x_sl": np.ascontiguousarray(x[:, sl, :]),
                "g_sl": np.ascontiguousarray(g[:, sl, :]),
                "m_in": m.reshape(B, 1),
                "dom_in": dom_f,
                "pmi_in": pmi,
            }
        )
    res = run_bass_kernel_spmd(nc, in_maps, core_ids=list(range(N_CORES)))
    LAST_RESULT = res
    out = np.empty((B, T, D), dtype=np.float32)
    for c in range(N_CORES):
        out[:, c * T_LOC : (c + 1) * T_LOC, :] = res.results[c]["out_sl"]
    return out
